# revision 10
# baseline (speedup 1.0000x reference)
"""Trainium2 Bass kernel for nn_AttentionBlock (GroupNorm + MHSA + proj + residual).

Data-parallel over batch: B=8 batch elements -> 8 NeuronCores, one each.
Per core (all in fp32, matmuls in fp32r):
  x_b [C=512, N=1024]
  group_norm (32 groups of 16 channels)
  qkv = W_qkv @ x_norm + b  (q,k in [c,n] layout; v computed directly transposed [n,c])
  per head h (8 heads, hd=64): S^T = k_h^T q_h  [m,n]; P = exp(S^T/8);
  out_h = v_h @ P / colsum (colsum via ones-column appended to v^T -> M=65 matmul)
  y = x + W_proj @ out + b_proj
"""

import numpy as np

C = 512
N = 1024  # H*W
NH = 8
HD = 64
NG = 32
EPS = 1e-5
NCORES = 8

_CACHE = {}


def _build_program():
    import concourse.bass as bass  # noqa: F401
    import concourse.mybir as mybir
    import concourse.tile as tile
    from concourse import bacc

    f32 = mybir.dt.float32
    bf16 = mybir.dt.bfloat16
    Act = mybir.ActivationFunctionType
    from concourse.alu_op_type import AluOpType as Op

    nc = bacc.Bacc("TRN2", target_bir_lowering=False, debug=False, num_devices=NCORES)

    # DRAM parameters (per-core shapes; same weights on all cores, x sharded by batch)
    x_d = nc.dram_tensor("x", [C, N], f32, kind="ExternalInput")
    wqkvT_d = nc.dram_tensor("wqkvT", [C, 3 * C], bf16, kind="ExternalInput")
    bqkv_d = nc.dram_tensor("bqkv", [1, 3 * C], bf16, kind="ExternalInput")
    wprojT_d = nc.dram_tensor("wprojT", [C, C], bf16, kind="ExternalInput")
    bproj_d = nc.dram_tensor("bproj", [1, C], bf16, kind="ExternalInput")
    gammaT_d = nc.dram_tensor("gammaT", [128, 4], f32, kind="ExternalInput")
    betaT_d = nc.dram_tensor("betaT", [128, 4], f32, kind="ExternalInput")
    gsel_d = nc.dram_tensor("gsel", [128, 4, NG], f32, kind="ExternalInput")
    bsel_d = nc.dram_tensor("bsel", [NG, C], f32, kind="ExternalInput")
    onesr_d = nc.dram_tensor("onesr", [1, 512], bf16, kind="ExternalInput")
    onescol_d = nc.dram_tensor("onescol", [128, NH], bf16, kind="ExternalInput")
    y_d = nc.dram_tensor("y", [C, N], f32, kind="ExternalOutput")

    with tile.TileContext(nc) as tc:
        with tc.tile_pool(name="mem", bufs=1) as mem:
            # ---- persistent tiles ----
            x_t = [mem.tile([128, N], f32, tag=f"x{k}", name=f"x{k}") for k in range(4)]
            xn_t = [mem.tile([128, N], bf16, tag=f"xn{k}", name=f"xn{k}") for k in range(4)]
            wq_t = [mem.tile([128, 3 * C], bf16, tag=f"wq{k}", name=f"wq{k}") for k in range(4)]
            wp_t = [mem.tile([128, C], bf16, tag=f"wp{k}", name=f"wp{k}") for k in range(4)]
            bq_t = mem.tile([1, 3 * C], bf16, tag="bq", name="bq")
            bp_t = mem.tile([1, C], bf16, tag="bp", name="bp")
            gam_t = mem.tile([128, 4], f32, tag="gam", name="gam")
            bet_t = mem.tile([128, 4], f32, tag="bet", name="bet")
            gsel_t = mem.tile([128, 4, NG], f32, tag="gsel", name="gsel")
            bsel_t = mem.tile([NG, C], f32, tag="bsel", name="bsel")
            ones_t = mem.tile([1, 512], bf16, tag="ones", name="ones")
            qk_t = [mem.tile([128, N], bf16, tag=f"qk{oc}", name=f"qk{oc}") for oc in range(8)]
            # v^T with a ones column appended per head: [128, 8 heads, 65]
            vp_t = [mem.tile([128, NH, HD + 1], bf16, tag=f"vp{mc}", name=f"vp{mc}") for mc in range(8)]
            out_t = [mem.tile([128, N], bf16, tag=f"out{hp}", name=f"out{hp}") for hp in range(4)]
            y_t = [mem.tile([128, N], f32, tag=f"y{k}", name=f"y{k}") for k in range(4)]

            for k in range(4):
                nc.sync.dma_start(out=x_t[k], in_=x_d[k * 128:(k + 1) * 128, :])
            nc.gpsimd.dma_start(out=gam_t, in_=gammaT_d[:])
            nc.gpsimd.dma_start(out=bet_t, in_=betaT_d[:])
            nc.gpsimd.dma_start(out=gsel_t, in_=gsel_d[:])
            nc.gpsimd.dma_start(out=bsel_t, in_=bsel_d[:])
            nc.gpsimd.dma_start(out=ones_t, in_=onesr_d[:])
            nc.gpsimd.dma_start(out=bq_t, in_=bqkv_d[:])
            nc.gpsimd.dma_start(out=bp_t, in_=bproj_d[:])
            for k in range(4):
                nc.sync.dma_start(out=wq_t[k], in_=wqkvT_d[k * 128:(k + 1) * 128, :])
            for k in range(4):
                nc.sync.dma_start(out=wp_t[k], in_=wprojT_d[k * 128:(k + 1) * 128, :])
            for mc in range(8):
                nc.gpsimd.dma_start(out=vp_t[mc][:, :, HD:HD + 1], in_=onescol_d[:, :, None])

            # ---- group norm ----
            with (
                tc.tile_pool(name="gn", bufs=1) as gn,
                tc.tile_pool(name="pgn", bufs=2, space="PSUM") as pgn,
            ):
                s2_t = []
                for k in range(4):
                    st = gn.tile([128, 2, 6], f32, tag=f"st{k}", name=f"st{k}")
                    for j in range(2):
                        nc.vector.bn_stats(out=st[:, j, :], in_=x_t[k][:, j * 512:(j + 1) * 512])
                    mv = gn.tile([128, 2], f32, tag=f"mv{k}", name=f"mv{k}")
                    nc.vector.bn_aggr(out=mv, in_=st)
                    s2 = gn.tile([128, 2], f32, tag=f"s2{k}", name=f"s2{k}")
                    nc.vector.tensor_copy(out=s2[:, 0:1], in_=mv[:, 0:1])
                    nc.vector.tensor_tensor(out=s2[:, 1:2], in0=mv[:, 0:1], in1=mv[:, 0:1], op=Op.mult)
                    nc.vector.tensor_tensor(out=s2[:, 1:2], in0=s2[:, 1:2], in1=mv[:, 1:2], op=Op.add)
                    s2_t.append(s2)
                mvps = pgn.tile([NG, 2], f32, tag="mvps", name="mvps")
                for k in range(4):
                    nc.tensor.matmul(mvps, gsel_t[:, k, :], s2_t[k], start=(k == 0), stop=(k == 3))
                gn2 = gn.tile([NG, 2], f32, tag="gn2", name="gn2")
                eps_t = gn.tile([NG, 1], f32, tag="eps", name="eps")
                nc.vector.memset(eps_t, EPS)
                nc.vector.tensor_copy(out=gn2, in_=mvps)
                gnv = gn.tile([NG, 1], f32, tag="gnv", name="gnv")
                nc.vector.tensor_tensor(out=gnv, in0=gn2[:, 0:1], in1=gn2[:, 0:1], op=Op.mult)
                nc.vector.tensor_tensor(out=gn2[:, 1:2], in0=gn2[:, 1:2], in1=gnv, op=Op.subtract)
                nc.scalar.activation(out=gn2[:, 1:2], in_=gn2[:, 1:2], func=Act.Sqrt, bias=eps_t, scale=1.0)
                nc.vector.reciprocal(out=gn2[:, 1:2], in_=gn2[:, 1:2])
                for k in range(4):
                    bcp = pgn.tile([128, 2], f32, tag="bcp", name="bcp")
                    nc.tensor.matmul(bcp, bsel_t[:, k * 128:(k + 1) * 128], gn2, start=True, stop=True)
                    sc = gn.tile([128, 1], f32, tag=f"sc{k}", name=f"sc{k}")
                    tcv = gn.tile([128, 1], f32, tag=f"tc{k}", name=f"tc{k}")
                    nc.vector.tensor_tensor(out=sc, in0=bcp[:, 1:2], in1=gam_t[:, k:k + 1], op=Op.mult)
                    nc.vector.tensor_tensor(out=tcv, in0=bcp[:, 0:1], in1=sc, op=Op.mult)
                    nc.vector.tensor_tensor(out=tcv, in0=bet_t[:, k:k + 1], in1=tcv, op=Op.subtract)
                    nc.vector.tensor_scalar(out=xn_t[k], in0=x_t[k], scalar1=sc, scalar2=tcv,
                                            op0=Op.mult, op1=Op.add)

            # ---- QKV groups + attention, interleaved so ACT (exp) starts early ----
            with (
                tc.tile_pool(name="att", bufs=2) as att,
                tc.tile_pool(name="pqkv", bufs=2, space="PSUM") as pqkv,
                tc.tile_pool(name="pS", bufs=2, space="PSUM") as pS,
                tc.tile_pool(name="pO", bufs=1, space="PSUM") as pO,
            ):
                def qk_group(oc, nt):
                    # q/k chunk oc (0-7 of 3C), n-half nt -> qk_t[oc][:, nt]
                    osl = slice(oc * 128, (oc + 1) * 128)
                    nsl = slice(nt * 512, (nt + 1) * 512)
                    ps = pqkv.tile([128, 512], f32, tag="qkv", name="qkv")
                    nc.tensor.matmul(ps, bq_t[0:1, osl], ones_t, start=True, stop=False)
                    for k in range(4):
                        nc.tensor.matmul(ps, wq_t[k][:, osl], xn_t[k][:, nsl],
                                         start=False, stop=(k == 3))
                    nc.vector.tensor_copy(out=qk_t[oc][:, nsl], in_=ps)

                def vt_group(mc):
                    # v^T m-chunk mc -> vp_t[mc][:, :, 0:HD]
                    msl = slice(mc * 128, (mc + 1) * 128)
                    ps = pqkv.tile([128, 512], f32, tag="qkv", name="qkv")
                    nc.tensor.matmul(ps, ones_t[0:1, 0:128], bq_t[0:1, 2 * C:3 * C], start=True, stop=False)
                    for k in range(4):
                        nc.tensor.matmul(ps, xn_t[k][:, msl], wq_t[k][:, 2 * C:3 * C],
                                         start=False, stop=(k == 3))
                    nc.vector.tensor_copy(
                        out=vp_t[mc][:, :, 0:HD],
                        in_=ps.rearrange("p (h c) -> p h c", h=NH),
                    )

                def proj_group(oc, nt):
                    osl = slice(oc * 128, (oc + 1) * 128)
                    nsl = slice(nt * 512, (nt + 1) * 512)
                    ps = pqkv.tile([128, 512], f32, tag="qkv", name="qkvp")
                    nc.tensor.matmul(ps, bp_t[0:1, osl], ones_t, start=True, stop=False)
                    for k in range(4):
                        nc.tensor.matmul(ps, wp_t[k][:, osl], out_t[k][:, nsl],
                                         start=False, stop=(k == 3))
                    nc.vector.tensor_tensor(out=y_t[oc][:, nsl], in0=ps,
                                            in1=x_t[oc][:, nsl], op=Op.add)
                    nc.sync.dma_start(out=y_d[oc * 128:(oc + 1) * 128, nsl], in_=y_t[oc][:, nsl])

                # BISECT: emit all qkv groups upfront
                for mc in range(8):
                    vt_group(mc)
                for oc in range(8):
                    for nt in range(2):
                        qk_group(oc, nt)
                pending_DISABLED = True
                for g in []:
                    g()

                # remaining qkv groups, emitted just-in-time inside the attention loop
                pending = {0: [lambda: vt_group(2)], 1: [lambda: vt_group(3)],
                           2: [lambda: vt_group(4)], 3: [lambda: vt_group(5)],
                           4: [lambda: vt_group(6)], 5: [lambda: vt_group(7)],
                           6: [lambda: qk_group(1, 0)], 7: [lambda: qk_group(5, 0)],
                           10: [lambda: qk_group(2, 0)], 12: [lambda: qk_group(6, 0)],
                           14: [lambda: qk_group(3, 0)], 16: [lambda: qk_group(7, 0)],
                           18: [lambda: qk_group(0, 1)], 19: [lambda: qk_group(4, 1)],
                           20: [lambda: qk_group(1, 1)], 21: [lambda: qk_group(5, 1)],
                           22: [lambda: qk_group(2, 1)], 23: [lambda: qk_group(6, 1)],
                           24: [lambda: qk_group(3, 1)], 25: [lambda: qk_group(7, 1)]}

                it = 0
                for nt in range(2):
                    nsl = slice(nt * 512, (nt + 1) * 512)
                    for hp in range(4):
                        if nt == 1 and hp >= 1:
                            proj_group(hp - 1, 0)
                        q_t = qk_t[hp]
                        k_t = qk_t[4 + hp]
                        outA = pO.tile([HD + 1, 512], f32, tag="outA", name="outA")
                        outB = pO.tile([HD + 1, 512], f32, tag="outB", name="outB")
                        for mc in range(8):
                            for g in pending.pop(it, []) if False else []:
                                g()
                            it += 1
                            msl = slice(mc * 128, (mc + 1) * 128)
                            Sps = pS.tile([128, 1024], f32, tag="S", name="S")
                            nc.tensor.matmul(Sps[:, 0:512], k_t[0:64, msl], q_t[0:64, nsl],
                                             start=True, stop=True)
                            nc.tensor.matmul(Sps[:, 512:1024], k_t[64:128, msl], q_t[64:128, nsl],
                                             start=True, stop=True, tile_position=(64, 0))
                            ex = att.tile([128, 1024], bf16, tag="ex", name="ex")
                            nc.scalar.activation(out=ex, in_=Sps, func=Act.Exp, scale=0.125)
                            nc.tensor.matmul(outA, vp_t[mc][:, 2 * hp, :], ex[:, 0:512],
                                             start=(mc == 0), stop=(mc == 7))
                            nc.tensor.matmul(outB, vp_t[mc][:, 2 * hp + 1, :], ex[:, 512:1024],
                                             start=(mc == 0), stop=(mc == 7))
                        # copy AV outputs to SBUF immediately (frees PSUM), then normalize
                        oA = att.tile([65, 512], f32, tag="oA", name="oA")
                        oB = att.tile([65, 512], f32, tag="oB", name="oB")
                        nc.vector.tensor_copy(out=oA, in_=outA)
                        nc.vector.tensor_copy(out=oB, in_=outB)
                        rc2 = att.tile([1, 1024], f32, tag="rc2", name="rc2")
                        nc.gpsimd.dma_start(out=rc2[0:1, 0:512], in_=oA[64:65, :])
                        nc.gpsimd.dma_start(out=rc2[0:1, 512:1024], in_=oB[64:65, :])
                        rc2b = att.tile([1, 1024], f32, tag="rc2b", name="rc2b")
                        nc.vector.reciprocal_approx_fast(out=rc2b[0:1, :], in_=rc2[0:1, :])
                        bc2 = att.tile([64, 1024], f32, tag="bc2", name="bc2")
                        nc.gpsimd.partition_broadcast(bc2, rc2b[0:1, :])
                        nc.vector.tensor_tensor(out=out_t[hp][0:64, nsl], in0=oA[0:64, :],
                                                in1=bc2[:, 0:512], op=Op.mult)
                        stagB = att.tile([64, 512], bf16, tag="stagB", name="stagB")
                        nc.vector.tensor_tensor(out=stagB, in0=oB[0:64, :],
                                                in1=bc2[:, 512:1024], op=Op.mult)
                        nc.gpsimd.dma_start(out=out_t[hp][64:128, nsl], in_=stagB)
                # tail projections
                proj_group(3, 0)
                for oc in range(4):
                    proj_group(oc, 1)

    nc.compile()
    return nc


def _host_inputs(x, gamma, beta, w_qkv, b_qkv, w_proj, b_proj):
    import ml_dtypes
    f = np.float32
    bf = ml_dtypes.bfloat16
    xb = np.ascontiguousarray(np.asarray(x, f).reshape(NCORES, C, N))
    wqkvT = np.ascontiguousarray(np.asarray(w_qkv, f).T.astype(bf))     # [C, 3C]
    bq = np.ascontiguousarray(np.asarray(b_qkv, f)[None, :].astype(bf))
    wprojT = np.ascontiguousarray(np.asarray(w_proj, f).T.astype(bf))   # [C, C]
    bp = np.ascontiguousarray(np.asarray(b_proj, f)[None, :].astype(bf))
    gT = np.ascontiguousarray(np.asarray(gamma, f).reshape(4, 128).T)  # [128, 4]
    bT = np.ascontiguousarray(np.asarray(beta, f).reshape(4, 128).T)
    gsel = np.zeros((128, 4, NG), f)
    bsel = np.zeros((NG, C), f)
    for k in range(4):
        for p in range(128):
            g = 8 * k + p // 16
            gsel[p, k, g] = 1.0 / 16.0
            bsel[g, k * 128 + p] = 1.0
    onesr = np.ones((1, 512), bf)
    onescol = np.ones((128, NH), bf)
    shared = {"wqkvT": wqkvT, "bqkv": bq, "wprojT": wprojT, "bproj": bp,
              "gammaT": gT, "betaT": bT, "gsel": gsel, "bsel": bsel,
              "onesr": onesr, "onescol": onescol}
    return [dict(shared, x=xb[i]) for i in range(NCORES)]


def run(inputs, trace=False, **kwargs):
    from concourse.bass_utils import run_bass_kernel_spmd
    if "nc" not in _CACHE:
        _CACHE["nc"] = _build_program()
    nc = _CACHE["nc"]
    in_maps = _host_inputs(**inputs)
    res = run_bass_kernel_spmd(nc, in_maps, core_ids=list(range(NCORES)), trace=trace, **kwargs)
    B = inputs["x"].shape[0]
    H = W = 32
    y = np.stack([res.results[i]["y"].reshape(C, H, W) for i in range(NCORES)])
    return y.astype(np.float32), res


def kernel(**inputs):
    y, _ = run(inputs, trace=False)
    return y


# revision 11
# speedup vs baseline: 1.0517x; 1.0517x over previous
"""Trainium2 Bass kernel for nn_AttentionBlock (GroupNorm + MHSA + proj + residual).

Data-parallel over batch: B=8 batch elements -> 8 NeuronCores, one each.
Per core (all in fp32, matmuls in fp32r):
  x_b [C=512, N=1024]
  group_norm (32 groups of 16 channels)
  qkv = W_qkv @ x_norm + b  (q,k in [c,n] layout; v computed directly transposed [n,c])
  per head h (8 heads, hd=64): S^T = k_h^T q_h  [m,n]; P = exp(S^T/8);
  out_h = v_h @ P / colsum (colsum via ones-column appended to v^T -> M=65 matmul)
  y = x + W_proj @ out + b_proj
"""

import numpy as np

C = 512
N = 1024  # H*W
NH = 8
HD = 64
NG = 32
EPS = 1e-5
NCORES = 8

_CACHE = {}


def _build_program():
    import concourse.bass as bass  # noqa: F401
    import concourse.mybir as mybir
    import concourse.tile as tile
    from concourse import bacc

    f32 = mybir.dt.float32
    bf16 = mybir.dt.bfloat16
    Act = mybir.ActivationFunctionType
    from concourse.alu_op_type import AluOpType as Op

    nc = bacc.Bacc("TRN2", target_bir_lowering=False, debug=False, num_devices=NCORES)

    # DRAM parameters (per-core shapes; same weights on all cores, x sharded by batch)
    x_d = nc.dram_tensor("x", [C, N], f32, kind="ExternalInput")
    wqkvT_d = nc.dram_tensor("wqkvT", [C, 3 * C], bf16, kind="ExternalInput")
    bqkv_d = nc.dram_tensor("bqkv", [1, 3 * C], bf16, kind="ExternalInput")
    wprojT_d = nc.dram_tensor("wprojT", [C, C], bf16, kind="ExternalInput")
    bproj_d = nc.dram_tensor("bproj", [1, C], bf16, kind="ExternalInput")
    gammaT_d = nc.dram_tensor("gammaT", [128, 4], f32, kind="ExternalInput")
    betaT_d = nc.dram_tensor("betaT", [128, 4], f32, kind="ExternalInput")
    gsel_d = nc.dram_tensor("gsel", [128, 4, NG], f32, kind="ExternalInput")
    bsel_d = nc.dram_tensor("bsel", [NG, C], f32, kind="ExternalInput")
    onesr_d = nc.dram_tensor("onesr", [1, 512], bf16, kind="ExternalInput")
    onescol_d = nc.dram_tensor("onescol", [128, NH], bf16, kind="ExternalInput")
    y_d = nc.dram_tensor("y", [C, N], f32, kind="ExternalOutput")

    with tile.TileContext(nc) as tc:
        with tc.tile_pool(name="mem", bufs=1) as mem:
            # ---- persistent tiles ----
            x_t = [mem.tile([128, N], f32, tag=f"x{k}", name=f"x{k}") for k in range(4)]
            xn_t = [mem.tile([128, N], bf16, tag=f"xn{k}", name=f"xn{k}") for k in range(4)]
            wq_t = [mem.tile([128, 3 * C], bf16, tag=f"wq{k}", name=f"wq{k}") for k in range(4)]
            wp_t = [mem.tile([128, C], bf16, tag=f"wp{k}", name=f"wp{k}") for k in range(4)]
            bq_t = mem.tile([1, 3 * C], bf16, tag="bq", name="bq")
            bp_t = mem.tile([1, C], bf16, tag="bp", name="bp")
            gam_t = mem.tile([128, 4], f32, tag="gam", name="gam")
            bet_t = mem.tile([128, 4], f32, tag="bet", name="bet")
            gsel_t = mem.tile([128, 4, NG], f32, tag="gsel", name="gsel")
            bsel_t = mem.tile([NG, C], f32, tag="bsel", name="bsel")
            ones_t = mem.tile([1, 512], bf16, tag="ones", name="ones")
            qk_t = [mem.tile([128, N], bf16, tag=f"qk{oc}", name=f"qk{oc}") for oc in range(8)]
            # v^T with a ones column appended per head: [128, 8 heads, 65]
            vp_t = [mem.tile([128, NH, HD + 1], bf16, tag=f"vp{mc}", name=f"vp{mc}") for mc in range(8)]
            out_t = [mem.tile([128, N], bf16, tag=f"out{hp}", name=f"out{hp}") for hp in range(4)]
            y_t = [mem.tile([128, N], f32, tag=f"y{k}", name=f"y{k}") for k in range(4)]

            for k in range(4):
                nc.sync.dma_start(out=x_t[k], in_=x_d[k * 128:(k + 1) * 128, :])
            nc.gpsimd.dma_start(out=gam_t, in_=gammaT_d[:])
            nc.gpsimd.dma_start(out=bet_t, in_=betaT_d[:])
            nc.gpsimd.dma_start(out=gsel_t, in_=gsel_d[:])
            nc.gpsimd.dma_start(out=bsel_t, in_=bsel_d[:])
            nc.gpsimd.dma_start(out=ones_t, in_=onesr_d[:])
            nc.gpsimd.dma_start(out=bq_t, in_=bqkv_d[:])
            nc.gpsimd.dma_start(out=bp_t, in_=bproj_d[:])
            for k in range(4):
                nc.sync.dma_start(out=wq_t[k], in_=wqkvT_d[k * 128:(k + 1) * 128, :])
            for k in range(4):
                nc.sync.dma_start(out=wp_t[k], in_=wprojT_d[k * 128:(k + 1) * 128, :])
            for mc in range(8):
                nc.gpsimd.dma_start(out=vp_t[mc][:, :, HD:HD + 1], in_=onescol_d[:, :, None])

            # ---- group norm ----
            with (
                tc.tile_pool(name="gn", bufs=1) as gn,
                tc.tile_pool(name="pgn", bufs=2, space="PSUM") as pgn,
            ):
                s2_t = []
                for k in range(4):
                    st = gn.tile([128, 2, 6], f32, tag=f"st{k}", name=f"st{k}")
                    for j in range(2):
                        nc.vector.bn_stats(out=st[:, j, :], in_=x_t[k][:, j * 512:(j + 1) * 512])
                    mv = gn.tile([128, 2], f32, tag=f"mv{k}", name=f"mv{k}")
                    nc.vector.bn_aggr(out=mv, in_=st)
                    s2 = gn.tile([128, 2], f32, tag=f"s2{k}", name=f"s2{k}")
                    nc.vector.tensor_copy(out=s2[:, 0:1], in_=mv[:, 0:1])
                    nc.vector.tensor_tensor(out=s2[:, 1:2], in0=mv[:, 0:1], in1=mv[:, 0:1], op=Op.mult)
                    nc.vector.tensor_tensor(out=s2[:, 1:2], in0=s2[:, 1:2], in1=mv[:, 1:2], op=Op.add)
                    s2_t.append(s2)
                mvps = pgn.tile([NG, 2], f32, tag="mvps", name="mvps")
                for k in range(4):
                    nc.tensor.matmul(mvps, gsel_t[:, k, :], s2_t[k], start=(k == 0), stop=(k == 3))
                gn2 = gn.tile([NG, 2], f32, tag="gn2", name="gn2")
                eps_t = gn.tile([NG, 1], f32, tag="eps", name="eps")
                nc.vector.memset(eps_t, EPS)
                nc.vector.tensor_copy(out=gn2, in_=mvps)
                gnv = gn.tile([NG, 1], f32, tag="gnv", name="gnv")
                nc.vector.tensor_tensor(out=gnv, in0=gn2[:, 0:1], in1=gn2[:, 0:1], op=Op.mult)
                nc.vector.tensor_tensor(out=gn2[:, 1:2], in0=gn2[:, 1:2], in1=gnv, op=Op.subtract)
                nc.scalar.activation(out=gn2[:, 1:2], in_=gn2[:, 1:2], func=Act.Sqrt, bias=eps_t, scale=1.0)
                nc.vector.reciprocal(out=gn2[:, 1:2], in_=gn2[:, 1:2])
                for k in range(4):
                    bcp = pgn.tile([128, 2], f32, tag="bcp", name="bcp")
                    nc.tensor.matmul(bcp, bsel_t[:, k * 128:(k + 1) * 128], gn2, start=True, stop=True)
                    sc = gn.tile([128, 1], f32, tag=f"sc{k}", name=f"sc{k}")
                    tcv = gn.tile([128, 1], f32, tag=f"tc{k}", name=f"tc{k}")
                    nc.vector.tensor_tensor(out=sc, in0=bcp[:, 1:2], in1=gam_t[:, k:k + 1], op=Op.mult)
                    nc.vector.tensor_tensor(out=tcv, in0=bcp[:, 0:1], in1=sc, op=Op.mult)
                    nc.vector.tensor_tensor(out=tcv, in0=bet_t[:, k:k + 1], in1=tcv, op=Op.subtract)
                    nc.vector.tensor_scalar(out=xn_t[k], in0=x_t[k], scalar1=sc, scalar2=tcv,
                                            op0=Op.mult, op1=Op.add)

            # ---- QKV groups + attention, interleaved so ACT (exp) starts early ----
            with (
                tc.tile_pool(name="att", bufs=2) as att,
                tc.tile_pool(name="pqkv", bufs=2, space="PSUM") as pqkv,
                tc.tile_pool(name="pS", bufs=2, space="PSUM") as pS,
                tc.tile_pool(name="pO", bufs=1, space="PSUM") as pO,
            ):
                def qk_group(oc, nt):
                    # q/k chunk oc (0-7 of 3C), n-half nt -> qk_t[oc][:, nt]
                    osl = slice(oc * 128, (oc + 1) * 128)
                    nsl = slice(nt * 512, (nt + 1) * 512)
                    ps = pqkv.tile([128, 512], f32, tag="qkv", name="qkv")
                    nc.tensor.matmul(ps, bq_t[0:1, osl], ones_t, start=True, stop=False)
                    for k in range(4):
                        nc.tensor.matmul(ps, wq_t[k][:, osl], xn_t[k][:, nsl],
                                         start=False, stop=(k == 3))
                    nc.vector.tensor_copy(out=qk_t[oc][:, nsl], in_=ps)

                def vt_group(mc):
                    # v^T m-chunk mc -> vp_t[mc][:, :, 0:HD]
                    msl = slice(mc * 128, (mc + 1) * 128)
                    ps = pqkv.tile([128, 512], f32, tag="qkv", name="qkv")
                    nc.tensor.matmul(ps, ones_t[0:1, 0:128], bq_t[0:1, 2 * C:3 * C], start=True, stop=False)
                    for k in range(4):
                        nc.tensor.matmul(ps, xn_t[k][:, msl], wq_t[k][:, 2 * C:3 * C],
                                         start=False, stop=(k == 3))
                    nc.vector.tensor_copy(
                        out=vp_t[mc][:, :, 0:HD],
                        in_=ps.rearrange("p (h c) -> p h c", h=NH),
                    )

                def proj_group(oc, nt):
                    osl = slice(oc * 128, (oc + 1) * 128)
                    nsl = slice(nt * 512, (nt + 1) * 512)
                    ps = pqkv.tile([128, 512], f32, tag="qkv", name="qkvp")
                    nc.tensor.matmul(ps, bp_t[0:1, osl], ones_t, start=True, stop=False)
                    for k in range(4):
                        nc.tensor.matmul(ps, wp_t[k][:, osl], out_t[k][:, nsl],
                                         start=False, stop=(k == 3))
                    nc.vector.tensor_tensor(out=y_t[oc][:, nsl], in0=ps,
                                            in1=x_t[oc][:, nsl], op=Op.add)
                    nc.sync.dma_start(out=y_d[oc * 128:(oc + 1) * 128, nsl], in_=y_t[oc][:, nsl])

                # prologue groups + all nt=1 halves upfront; interleave the rest
                for g in [lambda: vt_group(0), lambda: vt_group(1),
                          lambda: qk_group(0, 0), lambda: qk_group(4, 0)]:
                    g()
                for oc in range(8):
                    qk_group(oc, 1)

                # remaining qkv groups, emitted just-in-time inside the attention loop
                pending = {0: [lambda: vt_group(2)], 1: [lambda: vt_group(3)],
                           2: [lambda: vt_group(4)], 3: [lambda: vt_group(5)],
                           4: [lambda: vt_group(6)], 5: [lambda: vt_group(7)],
                           6: [lambda: qk_group(1, 0)], 7: [lambda: qk_group(5, 0)],
                           10: [lambda: qk_group(2, 0)], 12: [lambda: qk_group(6, 0)],
                           14: [lambda: qk_group(3, 0)], 16: [lambda: qk_group(7, 0)]}

                it = 0
                for nt in range(2):
                    nsl = slice(nt * 512, (nt + 1) * 512)
                    for hp in range(4):
                        if nt == 1 and hp >= 1:
                            proj_group(hp - 1, 0)
                        q_t = qk_t[hp]
                        k_t = qk_t[4 + hp]
                        outA = pO.tile([HD + 1, 512], f32, tag="outA", name="outA")
                        outB = pO.tile([HD + 1, 512], f32, tag="outB", name="outB")
                        for mc in range(8):
                            for g in pending.pop(it, []):
                                g()
                            it += 1
                            msl = slice(mc * 128, (mc + 1) * 128)
                            Sps = pS.tile([128, 1024], f32, tag="S", name="S")
                            nc.tensor.matmul(Sps[:, 0:512], k_t[0:64, msl], q_t[0:64, nsl],
                                             start=True, stop=True)
                            nc.tensor.matmul(Sps[:, 512:1024], k_t[64:128, msl], q_t[64:128, nsl],
                                             start=True, stop=True, tile_position=(64, 0))
                            ex = att.tile([128, 1024], bf16, tag="ex", name="ex")
                            nc.scalar.activation(out=ex, in_=Sps, func=Act.Exp, scale=0.125)
                            nc.tensor.matmul(outA, vp_t[mc][:, 2 * hp, :], ex[:, 0:512],
                                             start=(mc == 0), stop=(mc == 7))
                            nc.tensor.matmul(outB, vp_t[mc][:, 2 * hp + 1, :], ex[:, 512:1024],
                                             start=(mc == 0), stop=(mc == 7))
                        # copy AV outputs to SBUF immediately (frees PSUM), then normalize
                        oA = att.tile([65, 512], f32, tag="oA", name="oA")
                        oB = att.tile([65, 512], f32, tag="oB", name="oB")
                        nc.vector.tensor_copy(out=oA, in_=outA)
                        nc.vector.tensor_copy(out=oB, in_=outB)
                        rc2 = att.tile([1, 1024], f32, tag="rc2", name="rc2")
                        nc.gpsimd.dma_start(out=rc2[0:1, 0:512], in_=oA[64:65, :])
                        nc.gpsimd.dma_start(out=rc2[0:1, 512:1024], in_=oB[64:65, :])
                        rc2b = att.tile([1, 1024], f32, tag="rc2b", name="rc2b")
                        nc.vector.reciprocal_approx_fast(out=rc2b[0:1, :], in_=rc2[0:1, :])
                        bc2 = att.tile([64, 1024], f32, tag="bc2", name="bc2")
                        nc.gpsimd.partition_broadcast(bc2, rc2b[0:1, :])
                        nc.vector.tensor_tensor(out=out_t[hp][0:64, nsl], in0=oA[0:64, :],
                                                in1=bc2[:, 0:512], op=Op.mult)
                        stagB = att.tile([64, 512], bf16, tag="stagB", name="stagB")
                        nc.vector.tensor_tensor(out=stagB, in0=oB[0:64, :],
                                                in1=bc2[:, 512:1024], op=Op.mult)
                        nc.gpsimd.dma_start(out=out_t[hp][64:128, nsl], in_=stagB)
                # tail projections
                proj_group(3, 0)
                for oc in range(4):
                    proj_group(oc, 1)

    nc.compile()
    return nc


def _host_inputs(x, gamma, beta, w_qkv, b_qkv, w_proj, b_proj):
    import ml_dtypes
    f = np.float32
    bf = ml_dtypes.bfloat16
    xb = np.ascontiguousarray(np.asarray(x, f).reshape(NCORES, C, N))
    wqkvT = np.ascontiguousarray(np.asarray(w_qkv, f).T.astype(bf))     # [C, 3C]
    bq = np.ascontiguousarray(np.asarray(b_qkv, f)[None, :].astype(bf))
    wprojT = np.ascontiguousarray(np.asarray(w_proj, f).T.astype(bf))   # [C, C]
    bp = np.ascontiguousarray(np.asarray(b_proj, f)[None, :].astype(bf))
    gT = np.ascontiguousarray(np.asarray(gamma, f).reshape(4, 128).T)  # [128, 4]
    bT = np.ascontiguousarray(np.asarray(beta, f).reshape(4, 128).T)
    gsel = np.zeros((128, 4, NG), f)
    bsel = np.zeros((NG, C), f)
    for k in range(4):
        for p in range(128):
            g = 8 * k + p // 16
            gsel[p, k, g] = 1.0 / 16.0
            bsel[g, k * 128 + p] = 1.0
    onesr = np.ones((1, 512), bf)
    onescol = np.ones((128, NH), bf)
    shared = {"wqkvT": wqkvT, "bqkv": bq, "wprojT": wprojT, "bproj": bp,
              "gammaT": gT, "betaT": bT, "gsel": gsel, "bsel": bsel,
              "onesr": onesr, "onescol": onescol}
    return [dict(shared, x=xb[i]) for i in range(NCORES)]


def run(inputs, trace=False, **kwargs):
    from concourse.bass_utils import run_bass_kernel_spmd
    if "nc" not in _CACHE:
        _CACHE["nc"] = _build_program()
    nc = _CACHE["nc"]
    in_maps = _host_inputs(**inputs)
    res = run_bass_kernel_spmd(nc, in_maps, core_ids=list(range(NCORES)), trace=trace, **kwargs)
    B = inputs["x"].shape[0]
    H = W = 32
    y = np.stack([res.results[i]["y"].reshape(C, H, W) for i in range(NCORES)])
    return y.astype(np.float32), res


def kernel(**inputs):
    y, _ = run(inputs, trace=False)
    return y


# revision 13
# speedup vs baseline: 1.0968x; 1.0429x over previous
"""Trainium2 Bass kernel for nn_AttentionBlock (GroupNorm + MHSA + proj + residual).

Data-parallel over batch: B=8 batch elements -> 8 NeuronCores, one each.
Per core (all in fp32, matmuls in fp32r):
  x_b [C=512, N=1024]
  group_norm (32 groups of 16 channels)
  qkv = W_qkv @ x_norm + b  (q,k in [c,n] layout; v computed directly transposed [n,c])
  per head h (8 heads, hd=64): S^T = k_h^T q_h  [m,n]; P = exp(S^T/8);
  out_h = v_h @ P / colsum (colsum via ones-column appended to v^T -> M=65 matmul)
  y = x + W_proj @ out + b_proj
"""

import numpy as np

C = 512
N = 1024  # H*W
NH = 8
HD = 64
NG = 32
EPS = 1e-5
NCORES = 8

_CACHE = {}


def _build_program():
    import concourse.bass as bass  # noqa: F401
    import concourse.mybir as mybir
    import concourse.tile as tile
    from concourse import bacc

    f32 = mybir.dt.float32
    bf16 = mybir.dt.bfloat16
    Act = mybir.ActivationFunctionType
    from concourse.alu_op_type import AluOpType as Op

    nc = bacc.Bacc("TRN2", target_bir_lowering=False, debug=False, num_devices=NCORES)

    # DRAM parameters (per-core shapes; same weights on all cores, x sharded by batch)
    x_d = nc.dram_tensor("x", [C, N], f32, kind="ExternalInput")
    wqkvT_d = nc.dram_tensor("wqkvT", [C, 3 * C], bf16, kind="ExternalInput")
    bqkv_d = nc.dram_tensor("bqkv", [1, 3 * C], bf16, kind="ExternalInput")
    wprojT_d = nc.dram_tensor("wprojT", [C, C], bf16, kind="ExternalInput")
    bproj_d = nc.dram_tensor("bproj", [1, C], bf16, kind="ExternalInput")
    gammaT_d = nc.dram_tensor("gammaT", [128, 4], f32, kind="ExternalInput")
    betaT_d = nc.dram_tensor("betaT", [128, 4], f32, kind="ExternalInput")
    gsel_d = nc.dram_tensor("gsel", [128, 4, NG], f32, kind="ExternalInput")
    bsel_d = nc.dram_tensor("bsel", [NG, C], f32, kind="ExternalInput")
    onesr_d = nc.dram_tensor("onesr", [1, 512], bf16, kind="ExternalInput")
    onescol_d = nc.dram_tensor("onescol", [128, NH], bf16, kind="ExternalInput")
    y_d = nc.dram_tensor("y", [C, N], f32, kind="ExternalOutput")

    with tile.TileContext(nc) as tc:
        with tc.tile_pool(name="mem", bufs=1) as mem:
            # ---- persistent tiles ----
            x_t = [mem.tile([128, N], f32, tag=f"x{k}", name=f"x{k}") for k in range(4)]
            xn_t = [mem.tile([128, N], bf16, tag=f"xn{k}", name=f"xn{k}") for k in range(4)]
            wq_t = [mem.tile([128, 3 * C], bf16, tag=f"wq{k}", name=f"wq{k}") for k in range(4)]
            wp_t = [mem.tile([128, C], bf16, tag=f"wp{k}", name=f"wp{k}") for k in range(4)]
            bq_t = mem.tile([1, 3 * C], bf16, tag="bq", name="bq")
            bp_t = mem.tile([1, C], bf16, tag="bp", name="bp")
            gam_t = mem.tile([128, 4], f32, tag="gam", name="gam")
            bet_t = mem.tile([128, 4], f32, tag="bet", name="bet")
            gsel_t = mem.tile([128, 4, NG], f32, tag="gsel", name="gsel")
            bsel_t = mem.tile([NG, C], f32, tag="bsel", name="bsel")
            ones_t = mem.tile([1, 512], bf16, tag="ones", name="ones")
            qh_t = [[mem.tile([128, 512], bf16, tag=f"q{oc}n{nt}", name=f"q{oc}n{nt}")
                     for oc in range(4)] for nt in range(2)]
            kf_t = [mem.tile([128, N], bf16, tag=f"kf{j}", name=f"kf{j}") for j in range(4)]
            # v^T with a ones column appended per head: [128, 8 heads, 65]
            vp_t = [mem.tile([128, NH, HD + 1], bf16, tag=f"vp{mc}", name=f"vp{mc}") for mc in range(8)]
            out_t = [mem.tile([128, N], bf16, tag=f"out{hp}", name=f"out{hp}") for hp in range(4)]
            y_t = [mem.tile([128, N], f32, tag=f"y{k}", name=f"y{k}") for k in range(4)]

            for k in range(4):
                nc.sync.dma_start(out=x_t[k], in_=x_d[k * 128:(k + 1) * 128, :])
            nc.gpsimd.dma_start(out=gam_t, in_=gammaT_d[:])
            nc.gpsimd.dma_start(out=bet_t, in_=betaT_d[:])
            nc.gpsimd.dma_start(out=gsel_t, in_=gsel_d[:])
            nc.gpsimd.dma_start(out=bsel_t, in_=bsel_d[:])
            nc.gpsimd.dma_start(out=ones_t, in_=onesr_d[:])
            nc.gpsimd.dma_start(out=bq_t, in_=bqkv_d[:])
            nc.gpsimd.dma_start(out=bp_t, in_=bproj_d[:])
            for k in range(4):
                nc.sync.dma_start(out=wq_t[k], in_=wqkvT_d[k * 128:(k + 1) * 128, :])
            for k in range(4):
                nc.sync.dma_start(out=wp_t[k], in_=wprojT_d[k * 128:(k + 1) * 128, :])
            for mc in range(8):
                nc.gpsimd.dma_start(out=vp_t[mc][:, :, HD:HD + 1], in_=onescol_d[:, :, None])

            # ---- group norm ----
            with (
                tc.tile_pool(name="gn", bufs=1) as gn,
                tc.tile_pool(name="pgn", bufs=2, space="PSUM") as pgn,
            ):
                s2_t = []
                for k in range(4):
                    st = gn.tile([128, 2, 6], f32, tag=f"st{k}", name=f"st{k}")
                    for j in range(2):
                        nc.vector.bn_stats(out=st[:, j, :], in_=x_t[k][:, j * 512:(j + 1) * 512])
                    mv = gn.tile([128, 2], f32, tag=f"mv{k}", name=f"mv{k}")
                    nc.vector.bn_aggr(out=mv, in_=st)
                    s2 = gn.tile([128, 2], f32, tag=f"s2{k}", name=f"s2{k}")
                    nc.vector.tensor_copy(out=s2[:, 0:1], in_=mv[:, 0:1])
                    nc.vector.tensor_tensor(out=s2[:, 1:2], in0=mv[:, 0:1], in1=mv[:, 0:1], op=Op.mult)
                    nc.vector.tensor_tensor(out=s2[:, 1:2], in0=s2[:, 1:2], in1=mv[:, 1:2], op=Op.add)
                    s2_t.append(s2)
                mvps = pgn.tile([NG, 2], f32, tag="mvps", name="mvps")
                for k in range(4):
                    nc.tensor.matmul(mvps, gsel_t[:, k, :], s2_t[k], start=(k == 0), stop=(k == 3))
                gn2 = gn.tile([NG, 2], f32, tag="gn2", name="gn2")
                eps_t = gn.tile([NG, 1], f32, tag="eps", name="eps")
                nc.vector.memset(eps_t, EPS)
                nc.vector.tensor_copy(out=gn2, in_=mvps)
                gnv = gn.tile([NG, 1], f32, tag="gnv", name="gnv")
                nc.vector.tensor_tensor(out=gnv, in0=gn2[:, 0:1], in1=gn2[:, 0:1], op=Op.mult)
                nc.vector.tensor_tensor(out=gn2[:, 1:2], in0=gn2[:, 1:2], in1=gnv, op=Op.subtract)
                nc.scalar.activation(out=gn2[:, 1:2], in_=gn2[:, 1:2], func=Act.Sqrt, bias=eps_t, scale=1.0)
                nc.vector.reciprocal(out=gn2[:, 1:2], in_=gn2[:, 1:2])
                for k in range(4):
                    bcp = pgn.tile([128, 2], f32, tag="bcp", name="bcp")
                    nc.tensor.matmul(bcp, bsel_t[:, k * 128:(k + 1) * 128], gn2, start=True, stop=True)
                    sc = gn.tile([128, 1], f32, tag=f"sc{k}", name=f"sc{k}")
                    tcv = gn.tile([128, 1], f32, tag=f"tc{k}", name=f"tc{k}")
                    nc.vector.tensor_tensor(out=sc, in0=bcp[:, 1:2], in1=gam_t[:, k:k + 1], op=Op.mult)
                    nc.vector.tensor_tensor(out=tcv, in0=bcp[:, 0:1], in1=sc, op=Op.mult)
                    nc.vector.tensor_tensor(out=tcv, in0=bet_t[:, k:k + 1], in1=tcv, op=Op.subtract)
                    nc.vector.tensor_scalar(out=xn_t[k], in0=x_t[k], scalar1=sc, scalar2=tcv,
                                            op0=Op.mult, op1=Op.add)

            # ---- QKV groups + attention, interleaved so ACT (exp) starts early ----
            with (
                tc.tile_pool(name="att", bufs=2) as att,
                tc.tile_pool(name="pqkv", bufs=2, space="PSUM") as pqkv,
                tc.tile_pool(name="pS", bufs=2, space="PSUM") as pS,
                tc.tile_pool(name="pO", bufs=1, space="PSUM") as pO,
            ):
                def qkv_mms(ps, osl, nsl):
                    nc.tensor.matmul(ps, bq_t[0:1, osl], ones_t, start=True, stop=False)
                    for k in range(4):
                        nc.tensor.matmul(ps, wq_t[k][:, osl], xn_t[k][:, nsl],
                                         start=False, stop=(k == 3))

                def q_group(j, nt):
                    # query chunk j (head pair j), n-half nt -> qh_t[nt][j]
                    ps = pqkv.tile([128, 512], f32, tag="qkv", name="qkv")
                    qkv_mms(ps, slice(j * 128, (j + 1) * 128), slice(nt * 512, (nt + 1) * 512))
                    nc.vector.tensor_copy(out=qh_t[nt][j], in_=ps)

                def k_group(j, nt):
                    # key chunk j (head pair j), n-half nt -> kf_t[j][:, nt-half]
                    ps = pqkv.tile([128, 512], f32, tag="qkv", name="qkv")
                    qkv_mms(ps, slice(C + j * 128, C + (j + 1) * 128), slice(nt * 512, (nt + 1) * 512))
                    nc.vector.tensor_copy(out=kf_t[j][:, nt * 512:(nt + 1) * 512], in_=ps)

                def vt_group(mc):
                    # v^T m-chunk mc -> vp_t[mc][:, :, 0:HD]
                    msl = slice(mc * 128, (mc + 1) * 128)
                    ps = pqkv.tile([128, 512], f32, tag="qkv", name="qkv")
                    nc.tensor.matmul(ps, ones_t[0:1, 0:128], bq_t[0:1, 2 * C:3 * C], start=True, stop=False)
                    for k in range(4):
                        nc.tensor.matmul(ps, xn_t[k][:, msl], wq_t[k][:, 2 * C:3 * C],
                                         start=False, stop=(k == 3))
                    nc.vector.tensor_copy(
                        out=vp_t[mc][:, :, 0:HD],
                        in_=ps.rearrange("p (h c) -> p h c", h=NH),
                    )

                def proj_group(oc, nt):
                    osl = slice(oc * 128, (oc + 1) * 128)
                    nsl = slice(nt * 512, (nt + 1) * 512)
                    ps = pqkv.tile([128, 512], f32, tag="qkv", name="qkvp")
                    nc.tensor.matmul(ps, bp_t[0:1, osl], ones_t, start=True, stop=False)
                    for k in range(4):
                        nc.tensor.matmul(ps, wp_t[k][:, osl], out_t[k][:, nsl],
                                         start=False, stop=(k == 3))
                    nc.vector.tensor_tensor(out=y_t[oc][:, nsl], in0=ps,
                                            in1=x_t[oc][:, nsl], op=Op.add)
                    nc.sync.dma_start(out=y_d[oc * 128:(oc + 1) * 128, nsl], in_=y_t[oc][:, nsl])

                # prologue groups + all nt=1 halves upfront; interleave the rest
                for g in [lambda: vt_group(0), lambda: vt_group(1), lambda: q_group(0, 0),
                          lambda: k_group(0, 0), lambda: k_group(0, 1)]:
                    g()

                # remaining qkv groups, emitted just-in-time inside the attention loop
                pending = {0: [lambda: vt_group(2)], 1: [lambda: vt_group(3)],
                           2: [lambda: vt_group(4)], 3: [lambda: vt_group(5)],
                           4: [lambda: vt_group(6)], 5: [lambda: vt_group(7)],
                           6: [lambda: k_group(1, 0), lambda: k_group(1, 1)],
                           7: [lambda: q_group(1, 0)],
                           9: [lambda: k_group(2, 0)], 11: [lambda: k_group(2, 1)],
                           13: [lambda: q_group(2, 0)],
                           15: [lambda: k_group(3, 0)], 17: [lambda: k_group(3, 1)],
                           19: [lambda: q_group(3, 0)],
                           21: [lambda: q_group(0, 1)], 23: [lambda: q_group(1, 1)],
                           25: [lambda: q_group(2, 1)], 27: [lambda: q_group(3, 1)]}

                it = 0
                for nt in range(2):
                    nsl = slice(nt * 512, (nt + 1) * 512)
                    for hp in range(4):
                        if nt == 1 and hp >= 1:
                            proj_group(hp - 1, 0)
                        q_t = qh_t[nt][hp]
                        k_t = kf_t[hp]
                        outA = pO.tile([HD + 1, 512], f32, tag="outA", name="outA")
                        outB = pO.tile([HD + 1, 512], f32, tag="outB", name="outB")
                        for mc in range(8):
                            for g in pending.pop(it, []):
                                g()
                            it += 1
                            msl = slice(mc * 128, (mc + 1) * 128)
                            Sps = pS.tile([128, 1024], f32, tag="S", name="S")
                            nc.tensor.matmul(Sps[:, 0:512], k_t[0:64, msl], q_t[0:64, :],
                                             start=True, stop=True)
                            nc.tensor.matmul(Sps[:, 512:1024], k_t[64:128, msl], q_t[64:128, :],
                                             start=True, stop=True, tile_position=(64, 0))
                            ex = att.tile([128, 1024], bf16, tag="ex", name="ex")
                            nc.scalar.activation(out=ex, in_=Sps, func=Act.Exp, scale=0.125)
                            nc.tensor.matmul(outA, vp_t[mc][:, 2 * hp, :], ex[:, 0:512],
                                             start=(mc == 0), stop=(mc == 7))
                            nc.tensor.matmul(outB, vp_t[mc][:, 2 * hp + 1, :], ex[:, 512:1024],
                                             start=(mc == 0), stop=(mc == 7))
                        # copy AV outputs to SBUF immediately (frees PSUM), then normalize
                        oA = att.tile([65, 512], f32, tag="oA", name="oA")
                        oB = att.tile([65, 512], f32, tag="oB", name="oB")
                        nc.vector.tensor_copy(out=oA, in_=outA)
                        nc.vector.tensor_copy(out=oB, in_=outB)
                        rc2 = att.tile([1, 1024], f32, tag="rc2", name="rc2")
                        nc.gpsimd.dma_start(out=rc2[0:1, 0:512], in_=oA[64:65, :])
                        nc.gpsimd.dma_start(out=rc2[0:1, 512:1024], in_=oB[64:65, :])
                        rc2b = att.tile([1, 1024], f32, tag="rc2b", name="rc2b")
                        nc.vector.reciprocal_approx_fast(out=rc2b[0:1, :], in_=rc2[0:1, :])
                        bc2 = att.tile([64, 1024], f32, tag="bc2", name="bc2")
                        nc.gpsimd.partition_broadcast(bc2, rc2b[0:1, :])
                        nc.vector.tensor_tensor(out=out_t[hp][0:64, nsl], in0=oA[0:64, :],
                                                in1=bc2[:, 0:512], op=Op.mult)
                        stagB = att.tile([64, 512], bf16, tag="stagB", name="stagB")
                        nc.vector.tensor_tensor(out=stagB, in0=oB[0:64, :],
                                                in1=bc2[:, 512:1024], op=Op.mult)
                        nc.gpsimd.dma_start(out=out_t[hp][64:128, nsl], in_=stagB)
                # tail projections
                proj_group(3, 0)
                for oc in range(4):
                    proj_group(oc, 1)

    nc.compile()
    return nc


def _host_inputs(x, gamma, beta, w_qkv, b_qkv, w_proj, b_proj):
    import ml_dtypes
    f = np.float32
    bf = ml_dtypes.bfloat16
    xb = np.ascontiguousarray(np.asarray(x, f).reshape(NCORES, C, N))
    wqkvT = np.ascontiguousarray(np.asarray(w_qkv, f).T.astype(bf))     # [C, 3C]
    bq = np.ascontiguousarray(np.asarray(b_qkv, f)[None, :].astype(bf))
    wprojT = np.ascontiguousarray(np.asarray(w_proj, f).T.astype(bf))   # [C, C]
    bp = np.ascontiguousarray(np.asarray(b_proj, f)[None, :].astype(bf))
    gT = np.ascontiguousarray(np.asarray(gamma, f).reshape(4, 128).T)  # [128, 4]
    bT = np.ascontiguousarray(np.asarray(beta, f).reshape(4, 128).T)
    gsel = np.zeros((128, 4, NG), f)
    bsel = np.zeros((NG, C), f)
    for k in range(4):
        for p in range(128):
            g = 8 * k + p // 16
            gsel[p, k, g] = 1.0 / 16.0
            bsel[g, k * 128 + p] = 1.0
    onesr = np.ones((1, 512), bf)
    onescol = np.ones((128, NH), bf)
    shared = {"wqkvT": wqkvT, "bqkv": bq, "wprojT": wprojT, "bproj": bp,
              "gammaT": gT, "betaT": bT, "gsel": gsel, "bsel": bsel,
              "onesr": onesr, "onescol": onescol}
    return [dict(shared, x=xb[i]) for i in range(NCORES)]


def run(inputs, trace=False, **kwargs):
    from concourse.bass_utils import run_bass_kernel_spmd
    if "nc" not in _CACHE:
        _CACHE["nc"] = _build_program()
    nc = _CACHE["nc"]
    in_maps = _host_inputs(**inputs)
    res = run_bass_kernel_spmd(nc, in_maps, core_ids=list(range(NCORES)), trace=trace, **kwargs)
    B = inputs["x"].shape[0]
    H = W = 32
    y = np.stack([res.results[i]["y"].reshape(C, H, W) for i in range(NCORES)])
    return y.astype(np.float32), res


def kernel(**inputs):
    y, _ = run(inputs, trace=False)
    return y


# revision 14
# speedup vs baseline: 1.1222x; 1.0231x over previous
"""Trainium2 Bass kernel for nn_AttentionBlock (GroupNorm + MHSA + proj + residual).

Data-parallel over batch: B=8 batch elements -> 8 NeuronCores, one each.
Per core (fp32 activations/psum, bf16 matmul operands):
  x_b [C=512, N=1024]
  group_norm (32 groups of 16 channels)
  qkv = W_qkv @ x_norm + b  (q,k in [c,n] layout; v computed directly transposed [m,c])
  per head h (8 heads, hd=64): S^T = k_h^T q_h  [m,n]; P = exp(S^T/8);
  out_h = v_h @ P / colsum (colsum via ones-column appended to v^T -> M=65 matmul)
  y = x + W_proj @ out + b_proj

The attention inner loop is software-pipelined (QK of iteration i+1 is emitted
before AV of iteration i so the PE FIFO never blocks behind the exp wait), and
the QKV projection groups are interleaved just-in-time into the attention
stream so the Scalar engine (exp, the bottleneck) starts early and stays fed.
"""

import numpy as np

C = 512
N = 1024  # H*W
NH = 8
HD = 64
NG = 32
EPS = 1e-5
NCORES = 8

_CACHE = {}


def _build_program():
    import concourse.bass as bass  # noqa: F401
    import concourse.mybir as mybir
    import concourse.tile as tile
    from concourse import bacc

    f32 = mybir.dt.float32
    bf16 = mybir.dt.bfloat16
    Act = mybir.ActivationFunctionType
    from concourse.alu_op_type import AluOpType as Op

    nc = bacc.Bacc("TRN2", target_bir_lowering=False, debug=False, num_devices=NCORES)

    x_d = nc.dram_tensor("x", [C, N], f32, kind="ExternalInput")
    wqkvT_d = nc.dram_tensor("wqkvT", [C, 3 * C], bf16, kind="ExternalInput")
    wprojT_d = nc.dram_tensor("wprojT", [C, C], bf16, kind="ExternalInput")
    bqT_d = nc.dram_tensor("bqT", [128, 8], f32, kind="ExternalInput")
    bpT_d = nc.dram_tensor("bpT", [128, 4], f32, kind="ExternalInput")
    bvb_d = nc.dram_tensor("bvb", [128, 512], f32, kind="ExternalInput")
    gammaT_d = nc.dram_tensor("gammaT", [128, 4], f32, kind="ExternalInput")
    betaT_d = nc.dram_tensor("betaT", [128, 4], f32, kind="ExternalInput")
    gsel_d = nc.dram_tensor("gsel", [128, 4, NG], f32, kind="ExternalInput")
    bsel_d = nc.dram_tensor("bsel", [NG, C], f32, kind="ExternalInput")
    onescol_d = nc.dram_tensor("onescol", [128, NH], bf16, kind="ExternalInput")
    y_d = nc.dram_tensor("y", [C, N], f32, kind="ExternalOutput")

    with tile.TileContext(nc) as tc:
        with tc.tile_pool(name="mem", bufs=1) as mem:
            # ---- persistent tiles ----
            x_t = [mem.tile([128, N], f32, tag=f"x{k}", name=f"x{k}") for k in range(4)]
            xn_t = [mem.tile([128, N], bf16, tag=f"xn{k}", name=f"xn{k}") for k in range(4)]
            wq_t = [mem.tile([128, 3 * C], bf16, tag=f"wq{k}", name=f"wq{k}") for k in range(4)]
            wp_t = [mem.tile([128, C], bf16, tag=f"wp{k}", name=f"wp{k}") for k in range(4)]
            bqT_t = mem.tile([128, 8], f32, tag="bqT", name="bqT")
            bpT_t = mem.tile([128, 4], f32, tag="bpT", name="bpT")
            bvb_t = mem.tile([128, 512], f32, tag="bvb", name="bvb")
            gam_t = mem.tile([128, 4], f32, tag="gam", name="gam")
            bet_t = mem.tile([128, 4], f32, tag="bet", name="bet")
            gsel_t = mem.tile([128, 4, NG], f32, tag="gsel", name="gsel")
            bsel_t = mem.tile([NG, C], f32, tag="bsel", name="bsel")
            # query halves per (nt, head-pair); key full tiles per head-pair
            qh_t = [[mem.tile([128, 512], bf16, tag=f"q{j}n{nt}", name=f"q{j}n{nt}")
                     for j in range(4)] for nt in range(2)]
            kf_t = [mem.tile([128, N], bf16, tag=f"kf{j}", name=f"kf{j}") for j in range(4)]
            # v^T with a ones column appended per head: [128, 8 heads, 65]
            vp_t = [mem.tile([128, NH, HD + 1], bf16, tag=f"vp{mc}", name=f"vp{mc}") for mc in range(8)]
            out_t = [mem.tile([128, N], bf16, tag=f"out{hp}", name=f"out{hp}") for hp in range(4)]
            y_t = [mem.tile([128, N], f32, tag=f"y{k}", name=f"y{k}") for k in range(4)]

            for k in range(4):
                nc.sync.dma_start(out=x_t[k], in_=x_d[k * 128:(k + 1) * 128, :])
            nc.gpsimd.dma_start(out=gam_t, in_=gammaT_d[:])
            nc.gpsimd.dma_start(out=bet_t, in_=betaT_d[:])
            nc.gpsimd.dma_start(out=gsel_t, in_=gsel_d[:])
            nc.gpsimd.dma_start(out=bsel_t, in_=bsel_d[:])
            nc.gpsimd.dma_start(out=bqT_t, in_=bqT_d[:])
            nc.gpsimd.dma_start(out=bpT_t, in_=bpT_d[:])
            nc.gpsimd.dma_start(out=bvb_t, in_=bvb_d[:])
            for k in range(4):
                nc.sync.dma_start(out=wq_t[k], in_=wqkvT_d[k * 128:(k + 1) * 128, :])
            for k in range(4):
                nc.sync.dma_start(out=wp_t[k], in_=wprojT_d[k * 128:(k + 1) * 128, :])
            for mc in range(8):
                nc.gpsimd.dma_start(out=vp_t[mc][:, :, HD:HD + 1], in_=onescol_d[:, :, None])

            # ---- group norm ----
            with (
                tc.tile_pool(name="gn", bufs=1) as gn,
                tc.tile_pool(name="pgn", bufs=2, space="PSUM") as pgn,
            ):
                s2_t = []
                for k in range(4):
                    st = gn.tile([128, 2, 6], f32, tag=f"st{k}", name=f"st{k}")
                    for j in range(2):
                        nc.vector.bn_stats(out=st[:, j, :], in_=x_t[k][:, j * 512:(j + 1) * 512])
                    mv = gn.tile([128, 2], f32, tag=f"mv{k}", name=f"mv{k}")
                    nc.vector.bn_aggr(out=mv, in_=st)
                    s2 = gn.tile([128, 2], f32, tag=f"s2{k}", name=f"s2{k}")
                    nc.vector.tensor_copy(out=s2[:, 0:1], in_=mv[:, 0:1])
                    nc.vector.tensor_tensor(out=s2[:, 1:2], in0=mv[:, 0:1], in1=mv[:, 0:1], op=Op.mult)
                    nc.vector.tensor_tensor(out=s2[:, 1:2], in0=s2[:, 1:2], in1=mv[:, 1:2], op=Op.add)
                    s2_t.append(s2)
                mvps = pgn.tile([NG, 2], f32, tag="mvps", name="mvps")
                for k in range(4):
                    nc.tensor.matmul(mvps, gsel_t[:, k, :], s2_t[k], start=(k == 0), stop=(k == 3))
                gn2 = gn.tile([NG, 2], f32, tag="gn2", name="gn2")
                eps_t = gn.tile([NG, 1], f32, tag="eps", name="eps")
                nc.vector.memset(eps_t, EPS)
                nc.vector.tensor_copy(out=gn2, in_=mvps)
                gnv = gn.tile([NG, 1], f32, tag="gnv", name="gnv")
                nc.vector.tensor_tensor(out=gnv, in0=gn2[:, 0:1], in1=gn2[:, 0:1], op=Op.mult)
                nc.vector.tensor_tensor(out=gn2[:, 1:2], in0=gn2[:, 1:2], in1=gnv, op=Op.subtract)
                nc.scalar.activation(out=gn2[:, 1:2], in_=gn2[:, 1:2], func=Act.Sqrt, bias=eps_t, scale=1.0)
                nc.vector.reciprocal(out=gn2[:, 1:2], in_=gn2[:, 1:2])
                for k in range(4):
                    bcp = pgn.tile([128, 2], f32, tag="bcp", name="bcp")
                    nc.tensor.matmul(bcp, bsel_t[:, k * 128:(k + 1) * 128], gn2, start=True, stop=True)
                    sc = gn.tile([128, 1], f32, tag=f"sc{k}", name=f"sc{k}")
                    tcv = gn.tile([128, 1], f32, tag=f"tc{k}", name=f"tc{k}")
                    nc.vector.tensor_tensor(out=sc, in0=bcp[:, 1:2], in1=gam_t[:, k:k + 1], op=Op.mult)
                    nc.vector.tensor_tensor(out=tcv, in0=bcp[:, 0:1], in1=sc, op=Op.mult)
                    nc.vector.tensor_tensor(out=tcv, in0=bet_t[:, k:k + 1], in1=tcv, op=Op.subtract)
                    nc.vector.tensor_scalar(out=xn_t[k], in0=x_t[k], scalar1=sc, scalar2=tcv,
                                            op0=Op.mult, op1=Op.add)

            # ---- QKV groups + software-pipelined attention + proj ----
            with (
                tc.tile_pool(name="att", bufs=2) as att,
                tc.tile_pool(name="pqkv", bufs=2, space="PSUM") as pqkv,
                tc.tile_pool(name="pS", bufs=2, space="PSUM") as pS,
                tc.tile_pool(name="pO", bufs=1, space="PSUM") as pO,
            ):
                def qkv_mms(ps, osl, nsl):
                    for k in range(4):
                        nc.tensor.matmul(ps, wq_t[k][:, osl], xn_t[k][:, nsl],
                                         start=(k == 0), stop=(k == 3))

                def q_group(j, nt):
                    ps = pqkv.tile([128, 512], f32, tag="qkv", name="qkv")
                    qkv_mms(ps, slice(j * 128, (j + 1) * 128), slice(nt * 512, (nt + 1) * 512))
                    nc.vector.tensor_scalar_add(out=qh_t[nt][j], in0=ps, scalar1=bqT_t[:, j:j + 1])

                def k_group(j, nt):
                    ps = pqkv.tile([128, 512], f32, tag="qkv", name="qkv")
                    qkv_mms(ps, slice(C + j * 128, C + (j + 1) * 128), slice(nt * 512, (nt + 1) * 512))
                    nc.vector.tensor_scalar_add(out=kf_t[j][:, nt * 512:(nt + 1) * 512], in0=ps,
                                                scalar1=bqT_t[:, 4 + j:5 + j])

                def vt_group(mc):
                    msl = slice(mc * 128, (mc + 1) * 128)
                    ps = pqkv.tile([128, 512], f32, tag="qkv", name="qkv")
                    for k in range(4):
                        nc.tensor.matmul(ps, xn_t[k][:, msl], wq_t[k][:, 2 * C:3 * C],
                                         start=(k == 0), stop=(k == 3))
                    nc.vector.tensor_tensor(out=vp_t[mc][:, :, 0:HD],
                                            in0=ps.rearrange("p (h c) -> p h c", h=NH),
                                            in1=bvb_t.rearrange("p (h c) -> p h c", h=NH),
                                            op=Op.add)

                def proj_group(oc, nt):
                    osl = slice(oc * 128, (oc + 1) * 128)
                    nsl = slice(nt * 512, (nt + 1) * 512)
                    ps = pqkv.tile([128, 512], f32, tag="qkv", name="qkvp")
                    for k in range(4):
                        nc.tensor.matmul(ps, wp_t[k][:, osl], out_t[k][:, nsl],
                                         start=(k == 0), stop=(k == 3))
                    nc.vector.scalar_tensor_tensor(out=y_t[oc][:, nsl], in0=ps,
                                                   scalar=bpT_t[:, oc:oc + 1],
                                                   in1=x_t[oc][:, nsl], op0=Op.add, op1=Op.add)
                    nc.sync.dma_start(out=y_d[oc * 128:(oc + 1) * 128, nsl], in_=y_t[oc][:, nsl])

                # prologue groups: just enough for attention iteration 0
                for g in [lambda: vt_group(0), lambda: vt_group(1), lambda: q_group(0, 0),
                          lambda: k_group(0, 0), lambda: k_group(0, 1)]:
                    g()

                # just-in-time emission schedule (iteration index -> qkv groups)
                pending = {0: [lambda: vt_group(2)], 1: [lambda: vt_group(3)],
                           2: [lambda: vt_group(4)], 3: [lambda: vt_group(5)],
                           4: [lambda: vt_group(6)], 5: [lambda: vt_group(7)],
                           6: [lambda: k_group(1, 0), lambda: k_group(1, 1)],
                           7: [lambda: q_group(1, 0)],
                           9: [lambda: k_group(2, 0)], 11: [lambda: k_group(2, 1)],
                           13: [lambda: q_group(2, 0)],
                           15: [lambda: k_group(3, 0)], 17: [lambda: k_group(3, 1)],
                           19: [lambda: q_group(3, 0)],
                           21: [lambda: q_group(0, 1)], 23: [lambda: q_group(1, 1)],
                           25: [lambda: q_group(2, 1)], 27: [lambda: q_group(3, 1)]}

                seq = [(nt, hp, mc) for nt in range(2) for hp in range(4) for mc in range(8)]

                def qk_pair(nt, hp, mc):
                    msl = slice(mc * 128, (mc + 1) * 128)
                    Sps = pS.tile([128, 1024], f32, tag="S", name="S")
                    nc.tensor.matmul(Sps[:, 0:512], kf_t[hp][0:64, msl], qh_t[nt][hp][0:64, :],
                                     start=True, stop=True)
                    nc.tensor.matmul(Sps[:, 512:1024], kf_t[hp][64:128, msl], qh_t[nt][hp][64:128, :],
                                     start=True, stop=True, tile_position=(64, 0))
                    return Sps

                cur_S = qk_pair(*seq[0])
                outA = outB = None
                for i, (nt, hp, mc) in enumerate(seq):
                    for g in pending.pop(i, []):
                        g()
                    ex = att.tile([128, 1024], bf16, tag="ex", name="ex")
                    nc.scalar.activation(out=ex, in_=cur_S, func=Act.Exp, scale=0.125)
                    if i + 1 < len(seq):
                        cur_S = qk_pair(*seq[i + 1])
                    if mc == 0:
                        outA = pO.tile([HD + 1, 512], f32, tag="outA", name="outA")
                        outB = pO.tile([HD + 1, 512], f32, tag="outB", name="outB")
                    nc.tensor.matmul(outA, vp_t[mc][:, 2 * hp, :], ex[:, 0:512],
                                     start=(mc == 0), stop=(mc == 7))
                    nc.tensor.matmul(outB, vp_t[mc][:, 2 * hp + 1, :], ex[:, 512:1024],
                                     start=(mc == 0), stop=(mc == 7))
                    if mc == 7:
                        # copy AV outputs to SBUF right away (frees PSUM), then normalize
                        nsl = slice(nt * 512, (nt + 1) * 512)
                        oA = att.tile([65, 512], f32, tag="oA", name="oA")
                        oB = att.tile([65, 512], f32, tag="oB", name="oB")
                        nc.vector.tensor_copy(out=oA, in_=outA)
                        nc.vector.tensor_copy(out=oB, in_=outB)
                        rc2 = att.tile([1, 1024], f32, tag="rc2", name="rc2")
                        nc.gpsimd.dma_start(out=rc2[0:1, 0:512], in_=oA[64:65, :])
                        nc.gpsimd.dma_start(out=rc2[0:1, 512:1024], in_=oB[64:65, :])
                        rc2b = att.tile([1, 1024], f32, tag="rc2b", name="rc2b")
                        nc.vector.reciprocal_approx_fast(out=rc2b[0:1, :], in_=rc2[0:1, :])
                        bc2 = att.tile([64, 1024], f32, tag="bc2", name="bc2")
                        nc.gpsimd.partition_broadcast(bc2, rc2b[0:1, :])
                        nc.vector.tensor_tensor(out=out_t[hp][0:64, nsl], in0=oA[0:64, :],
                                                in1=bc2[:, 0:512], op=Op.mult)
                        stagB = att.tile([64, 512], bf16, tag="stagB", name="stagB")
                        nc.vector.tensor_tensor(out=stagB, in0=oB[0:64, :],
                                                in1=bc2[:, 512:1024], op=Op.mult)
                        nc.gpsimd.dma_start(out=out_t[hp][64:128, nsl], in_=stagB)
                        # overlap proj(nt=0) with attention(nt=1)
                        if nt == 1 and hp <= 2:
                            proj_group(hp, 0)
                # tail projections
                proj_group(3, 0)
                for oc in range(4):
                    proj_group(oc, 1)

    nc.compile()
    return nc


def _host_inputs(x, gamma, beta, w_qkv, b_qkv, w_proj, b_proj):
    import ml_dtypes
    f = np.float32
    bf = ml_dtypes.bfloat16
    xb = np.ascontiguousarray(np.asarray(x, f).reshape(NCORES, C, N))
    wqkvT = np.ascontiguousarray(np.asarray(w_qkv, f).T.astype(bf))     # [C, 3C]
    wprojT = np.ascontiguousarray(np.asarray(w_proj, f).T.astype(bf))   # [C, C]
    bq = np.asarray(b_qkv, f)
    bqT = np.ascontiguousarray(bq[0:1024].reshape(8, 128).T)            # [128, 8]
    bpT = np.ascontiguousarray(np.asarray(b_proj, f).reshape(4, 128).T)  # [128, 4]
    bvb = np.ascontiguousarray(np.broadcast_to(bq[1024:1536][None, :], (128, 512)))
    gT = np.ascontiguousarray(np.asarray(gamma, f).reshape(4, 128).T)   # [128, 4]
    bT = np.ascontiguousarray(np.asarray(beta, f).reshape(4, 128).T)
    gsel = np.zeros((128, 4, NG), f)
    bsel = np.zeros((NG, C), f)
    for k in range(4):
        for p in range(128):
            g = 8 * k + p // 16
            gsel[p, k, g] = 1.0 / 16.0
            bsel[g, k * 128 + p] = 1.0
    onescol = np.ones((128, NH), bf)
    shared = {"wqkvT": wqkvT, "wprojT": wprojT, "bqT": bqT, "bpT": bpT, "bvb": bvb,
              "gammaT": gT, "betaT": bT, "gsel": gsel, "bsel": bsel, "onescol": onescol}
    return [dict(shared, x=xb[i]) for i in range(NCORES)]


def run(inputs, trace=False, **kwargs):
    from concourse.bass_utils import run_bass_kernel_spmd
    if "nc" not in _CACHE:
        _CACHE["nc"] = _build_program()
    nc = _CACHE["nc"]
    in_maps = _host_inputs(**inputs)
    res = run_bass_kernel_spmd(nc, in_maps, core_ids=list(range(NCORES)), trace=trace, **kwargs)
    H = W = 32
    y = np.stack([res.results[i]["y"].reshape(C, H, W) for i in range(NCORES)])
    return y.astype(np.float32), res


def kernel(**inputs):
    y, _ = run(inputs, trace=False)
    return y


# revision 16
# speedup vs baseline: 1.3661x; 1.2174x over previous
"""Trainium2 Bass kernel for nn_AttentionBlock (GroupNorm + MHSA + proj + residual).

Data-parallel over batch: B=8 batch elements -> 8 NeuronCores, one each.
Per core (fp32 activations/psum, bf16 matmul operands):
  x_b [C=512, N=1024]
  group_norm (32 groups of 16 channels)
  qkv = W_qkv @ x_norm + b  (q,k in [c,n] layout; v computed directly transposed [m,c])
  per head h (8 heads, hd=64): S^T = k_h^T q_h  [m,n]; P = exp(S^T/8);
  out_h = v_h @ P / colsum (colsum via ones-column appended to v^T -> M=65 matmul)
  y = x + W_proj @ out + b_proj

The attention inner loop is software-pipelined (QK of iteration i+1 is emitted
before AV of iteration i so the PE FIFO never blocks behind the exp wait), and
the QKV projection groups are interleaved just-in-time into the attention
stream so the Scalar engine (exp, the bottleneck) starts early and stays fed.
"""

import numpy as np

C = 512
N = 1024  # H*W
NH = 8
HD = 64
NG = 32
EPS = 1e-5
NCORES = 8

_CACHE = {}


def _build_program():
    import concourse.bass as bass  # noqa: F401
    import concourse.mybir as mybir
    import concourse.tile as tile
    from concourse import bacc

    f32 = mybir.dt.float32
    bf16 = mybir.dt.bfloat16
    Act = mybir.ActivationFunctionType
    from concourse.alu_op_type import AluOpType as Op

    nc = bacc.Bacc("TRN2", target_bir_lowering=False, debug=False, num_devices=NCORES)

    x_d = nc.dram_tensor("x", [C, N], f32, kind="ExternalInput")
    wqkvT_d = nc.dram_tensor("wqkvT", [C, 3 * C], bf16, kind="ExternalInput")
    wprojT_d = nc.dram_tensor("wprojT", [C, C], bf16, kind="ExternalInput")
    bqT_d = nc.dram_tensor("bqT", [128, 8], f32, kind="ExternalInput")
    bpT_d = nc.dram_tensor("bpT", [128, 4], f32, kind="ExternalInput")
    bvb_d = nc.dram_tensor("bvb", [128, 512], f32, kind="ExternalInput")
    gammaT_d = nc.dram_tensor("gammaT", [128, 4], f32, kind="ExternalInput")
    betaT_d = nc.dram_tensor("betaT", [128, 4], f32, kind="ExternalInput")
    gsel_d = nc.dram_tensor("gsel", [128, 4, NG], f32, kind="ExternalInput")
    bsel_d = nc.dram_tensor("bsel", [NG, C], f32, kind="ExternalInput")
    onescol_d = nc.dram_tensor("onescol", [128, NH], bf16, kind="ExternalInput")
    y_d = nc.dram_tensor("y", [C, N], mybir.dt.bfloat16, kind="ExternalOutput")

    with tile.TileContext(nc) as tc:
        with tc.tile_pool(name="mem", bufs=1) as mem:
            # ---- persistent tiles ----
            x_t = [mem.tile([128, N], f32, tag=f"x{k}", name=f"x{k}") for k in range(4)]
            xn_t = [mem.tile([128, N], bf16, tag=f"xn{k}", name=f"xn{k}") for k in range(4)]
            wq_t = [mem.tile([128, 3 * C], bf16, tag=f"wq{k}", name=f"wq{k}") for k in range(4)]
            wp_t = [mem.tile([128, C], bf16, tag=f"wp{k}", name=f"wp{k}") for k in range(4)]
            bqT_t = mem.tile([128, 8], f32, tag="bqT", name="bqT")
            bpT_t = mem.tile([128, 4], f32, tag="bpT", name="bpT")
            bvb_t = mem.tile([128, 512], f32, tag="bvb", name="bvb")
            gam_t = mem.tile([128, 4], f32, tag="gam", name="gam")
            bet_t = mem.tile([128, 4], f32, tag="bet", name="bet")
            gsel_t = mem.tile([128, 4, NG], f32, tag="gsel", name="gsel")
            bsel_t = mem.tile([NG, C], f32, tag="bsel", name="bsel")
            # query halves per (nt, head-pair); key full tiles per head-pair
            qh_t = [[mem.tile([128, 512], bf16, tag=f"q{j}n{nt}", name=f"q{j}n{nt}")
                     for j in range(4)] for nt in range(2)]
            kf_t = [mem.tile([128, N], bf16, tag=f"kf{j}", name=f"kf{j}") for j in range(4)]
            # v^T with a ones column appended per head: [128, 8 heads, 65]
            vp_t = [mem.tile([128, NH, HD + 1], bf16, tag=f"vp{mc}", name=f"vp{mc}") for mc in range(8)]
            out_t = [mem.tile([128, N], bf16, tag=f"out{hp}", name=f"out{hp}") for hp in range(4)]
            y_t = [mem.tile([128, N], bf16, tag=f"y{k}", name=f"y{k}") for k in range(4)]

            qs = [nc.sync, nc.scalar, nc.sync, nc.scalar]
            for k in range(4):
                qs[k].dma_start(out=x_t[k], in_=x_d[k * 128:(k + 1) * 128, :])
            nc.gpsimd.dma_start(out=gam_t, in_=gammaT_d[:])
            nc.gpsimd.dma_start(out=bet_t, in_=betaT_d[:])
            nc.gpsimd.dma_start(out=gsel_t, in_=gsel_d[:])
            nc.gpsimd.dma_start(out=bsel_t, in_=bsel_d[:])
            nc.gpsimd.dma_start(out=bqT_t, in_=bqT_d[:])
            nc.gpsimd.dma_start(out=bpT_t, in_=bpT_d[:])
            nc.gpsimd.dma_start(out=bvb_t, in_=bvb_d[:])
            for k in range(4):
                qs[k].dma_start(out=wq_t[k], in_=wqkvT_d[k * 128:(k + 1) * 128, :])
            for k in range(4):
                qs[k].dma_start(out=wp_t[k], in_=wprojT_d[k * 128:(k + 1) * 128, :])
            for mc in range(8):
                nc.gpsimd.dma_start(out=vp_t[mc][:, :, HD:HD + 1], in_=onescol_d[:, :, None])

            # ---- group norm ----
            with (
                tc.tile_pool(name="gn", bufs=1) as gn,
                tc.tile_pool(name="pgn", bufs=2, space="PSUM") as pgn,
            ):
                s2_t = []
                for k in range(4):
                    st = gn.tile([128, 2, 6], f32, tag=f"st{k}", name=f"st{k}")
                    for j in range(2):
                        nc.vector.bn_stats(out=st[:, j, :], in_=x_t[k][:, j * 512:(j + 1) * 512])
                    mv = gn.tile([128, 2], f32, tag=f"mv{k}", name=f"mv{k}")
                    nc.vector.bn_aggr(out=mv, in_=st)
                    s2 = gn.tile([128, 2], f32, tag=f"s2{k}", name=f"s2{k}")
                    nc.vector.tensor_copy(out=s2[:, 0:1], in_=mv[:, 0:1])
                    nc.vector.tensor_tensor(out=s2[:, 1:2], in0=mv[:, 0:1], in1=mv[:, 0:1], op=Op.mult)
                    nc.vector.tensor_tensor(out=s2[:, 1:2], in0=s2[:, 1:2], in1=mv[:, 1:2], op=Op.add)
                    s2_t.append(s2)
                mvps = pgn.tile([NG, 2], f32, tag="mvps", name="mvps")
                for k in range(4):
                    nc.tensor.matmul(mvps, gsel_t[:, k, :], s2_t[k], start=(k == 0), stop=(k == 3))
                gn2 = gn.tile([NG, 2], f32, tag="gn2", name="gn2")
                eps_t = gn.tile([NG, 1], f32, tag="eps", name="eps")
                nc.vector.memset(eps_t, EPS)
                nc.vector.tensor_copy(out=gn2, in_=mvps)
                gnv = gn.tile([NG, 1], f32, tag="gnv", name="gnv")
                nc.vector.tensor_tensor(out=gnv, in0=gn2[:, 0:1], in1=gn2[:, 0:1], op=Op.mult)
                nc.vector.tensor_tensor(out=gn2[:, 1:2], in0=gn2[:, 1:2], in1=gnv, op=Op.subtract)
                nc.scalar.activation(out=gn2[:, 1:2], in_=gn2[:, 1:2], func=Act.Sqrt, bias=eps_t, scale=1.0)
                gn2r = gn.tile([NG, 1], f32, tag="gn2r", name="gn2r")
                nc.vector.reciprocal_approx_fast(out=gn2r, in_=gn2[:, 1:2])
                nc.vector.tensor_copy(out=gn2[:, 1:2], in_=gn2r)
                for k in range(4):
                    bcp = pgn.tile([128, 2], f32, tag="bcp", name="bcp")
                    nc.tensor.matmul(bcp, bsel_t[:, k * 128:(k + 1) * 128], gn2, start=True, stop=True)
                    sc = gn.tile([128, 1], f32, tag=f"sc{k}", name=f"sc{k}")
                    tcv = gn.tile([128, 1], f32, tag=f"tc{k}", name=f"tc{k}")
                    nc.vector.tensor_tensor(out=sc, in0=bcp[:, 1:2], in1=gam_t[:, k:k + 1], op=Op.mult)
                    nc.vector.tensor_tensor(out=tcv, in0=bcp[:, 0:1], in1=sc, op=Op.mult)
                    nc.vector.tensor_tensor(out=tcv, in0=bet_t[:, k:k + 1], in1=tcv, op=Op.subtract)
                    nc.vector.tensor_scalar(out=xn_t[k], in0=x_t[k], scalar1=sc, scalar2=tcv,
                                            op0=Op.mult, op1=Op.add)

            # ---- QKV groups + software-pipelined attention + proj ----
            with (
                tc.tile_pool(name="att", bufs=2) as att,
                tc.tile_pool(name="pqkv", bufs=2, space="PSUM") as pqkv,
                tc.tile_pool(name="pS", bufs=2, space="PSUM") as pS,
                tc.tile_pool(name="pO", bufs=1, space="PSUM") as pO,
            ):
                def qkv_mms(ps, osl, nsl):
                    for k in range(4):
                        nc.tensor.matmul(ps, wq_t[k][:, osl], xn_t[k][:, nsl],
                                         start=(k == 0), stop=(k == 3))

                def q_group(j, nt):
                    ps = pqkv.tile([128, 512], f32, tag="qkv", name="qkv")
                    qkv_mms(ps, slice(j * 128, (j + 1) * 128), slice(nt * 512, (nt + 1) * 512))
                    nc.vector.tensor_scalar_add(out=qh_t[nt][j], in0=ps, scalar1=bqT_t[:, j:j + 1])

                def k_group(j, nt):
                    ps = pqkv.tile([128, 512], f32, tag="qkv", name="qkv")
                    qkv_mms(ps, slice(C + j * 128, C + (j + 1) * 128), slice(nt * 512, (nt + 1) * 512))
                    nc.vector.tensor_scalar_add(out=kf_t[j][:, nt * 512:(nt + 1) * 512], in0=ps,
                                                scalar1=bqT_t[:, 4 + j:5 + j])

                def vt_group(mc):
                    msl = slice(mc * 128, (mc + 1) * 128)
                    ps = pqkv.tile([128, 512], f32, tag="qkv", name="qkv")
                    for k in range(4):
                        nc.tensor.matmul(ps, xn_t[k][:, msl], wq_t[k][:, 2 * C:3 * C],
                                         start=(k == 0), stop=(k == 3))
                    nc.vector.tensor_tensor(out=vp_t[mc][:, :, 0:HD],
                                            in0=ps.rearrange("p (h c) -> p h c", h=NH),
                                            in1=bvb_t.rearrange("p (h c) -> p h c", h=NH),
                                            op=Op.add)

                def proj_group(oc, nt):
                    osl = slice(oc * 128, (oc + 1) * 128)
                    nsl = slice(nt * 512, (nt + 1) * 512)
                    ps = pqkv.tile([128, 512], f32, tag="qkv", name="qkvp")
                    for k in range(4):
                        nc.tensor.matmul(ps, wp_t[k][:, osl], out_t[k][:, nsl],
                                         start=(k == 0), stop=(k == 3))
                    nc.vector.scalar_tensor_tensor(out=y_t[oc][:, nsl], in0=ps,
                                                   scalar=bpT_t[:, oc:oc + 1],
                                                   in1=x_t[oc][:, nsl], op0=Op.add, op1=Op.add)
                    nc.sync.dma_start(out=y_d[oc * 128:(oc + 1) * 128, nsl], in_=y_t[oc][:, nsl])

                # prologue groups: just enough for attention iteration 0
                for g in [lambda: vt_group(0), lambda: vt_group(1), lambda: q_group(0, 0),
                          lambda: k_group(0, 0), lambda: k_group(0, 1)]:
                    g()

                # just-in-time emission schedule (iteration index -> qkv groups)
                pending = {0: [lambda: vt_group(2)], 1: [lambda: vt_group(3)],
                           2: [lambda: vt_group(4)], 3: [lambda: vt_group(5)],
                           4: [lambda: vt_group(6)], 5: [lambda: vt_group(7)],
                           6: [lambda: k_group(1, 0), lambda: k_group(1, 1)],
                           7: [lambda: q_group(1, 0)],
                           9: [lambda: k_group(2, 0)], 11: [lambda: k_group(2, 1)],
                           13: [lambda: q_group(2, 0)],
                           15: [lambda: k_group(3, 0)], 17: [lambda: k_group(3, 1)],
                           19: [lambda: q_group(3, 0)],
                           28: [lambda: q_group(0, 1)], 34: [lambda: q_group(1, 1)],
                           42: [lambda: q_group(2, 1)], 50: [lambda: q_group(3, 1)]}

                seq = [(nt, hp, mc) for nt in range(2) for hp in range(4) for mc in range(8)]

                def qk_pair(nt, hp, mc):
                    msl = slice(mc * 128, (mc + 1) * 128)
                    Sps = pS.tile([128, 1024], f32, tag="S", name="S")
                    nc.tensor.matmul(Sps[:, 0:512], kf_t[hp][0:64, msl], qh_t[nt][hp][0:64, :],
                                     start=True, stop=True)
                    nc.tensor.matmul(Sps[:, 512:1024], kf_t[hp][64:128, msl], qh_t[nt][hp][64:128, :],
                                     start=True, stop=True, tile_position=(64, 0))
                    return Sps

                cur_S = qk_pair(*seq[0])
                outA = outB = None
                for i, (nt, hp, mc) in enumerate(seq):
                    for g in pending.pop(i, []):
                        g()
                    ex = att.tile([128, 1024], bf16, tag="ex", name="ex")
                    nc.scalar.activation(out=ex, in_=cur_S, func=Act.Exp, scale=0.125)
                    if i + 1 < len(seq):
                        cur_S = qk_pair(*seq[i + 1])
                    if mc == 0:
                        outA = pO.tile([HD + 1, 512], f32, tag="outA", name="outA")
                        outB = pO.tile([HD + 1, 512], f32, tag="outB", name="outB")
                    nc.tensor.matmul(outA, vp_t[mc][:, 2 * hp, :], ex[:, 0:512],
                                     start=(mc == 0), stop=(mc == 7))
                    nc.tensor.matmul(outB, vp_t[mc][:, 2 * hp + 1, :], ex[:, 512:1024],
                                     start=(mc == 0), stop=(mc == 7))
                    if mc == 7:
                        # copy AV outputs to SBUF right away (frees PSUM), then normalize
                        nsl = slice(nt * 512, (nt + 1) * 512)
                        oA = att.tile([65, 512], f32, tag="oA", name="oA")
                        oB = att.tile([65, 512], f32, tag="oB", name="oB")
                        nc.vector.tensor_copy(out=oA, in_=outA)
                        nc.vector.tensor_copy(out=oB, in_=outB)
                        rc2 = att.tile([1, 1024], f32, tag="rc2", name="rc2")
                        nc.gpsimd.dma_start(out=rc2[0:1, 0:512], in_=oA[64:65, :])
                        nc.gpsimd.dma_start(out=rc2[0:1, 512:1024], in_=oB[64:65, :])
                        rc2b = att.tile([1, 1024], f32, tag="rc2b", name="rc2b")
                        nc.vector.reciprocal_approx_fast(out=rc2b[0:1, :], in_=rc2[0:1, :])
                        bc2 = att.tile([64, 1024], f32, tag="bc2", name="bc2")
                        nc.gpsimd.partition_broadcast(bc2, rc2b[0:1, :])
                        nc.vector.tensor_tensor(out=out_t[hp][0:64, nsl], in0=oA[0:64, :],
                                                in1=bc2[:, 0:512], op=Op.mult)
                        stagB = att.tile([64, 512], bf16, tag="stagB", name="stagB")
                        nc.vector.tensor_tensor(out=stagB, in0=oB[0:64, :],
                                                in1=bc2[:, 512:1024], op=Op.mult)
                        nc.gpsimd.dma_start(out=out_t[hp][64:128, nsl], in_=stagB)
                        # overlap proj(nt=0) with attention(nt=1)
                        if nt == 1 and hp <= 2:
                            proj_group(hp, 0)
                # tail projections
                proj_group(3, 0)
                for oc in range(4):
                    proj_group(oc, 1)

    nc.compile()
    return nc


def _host_inputs(x, gamma, beta, w_qkv, b_qkv, w_proj, b_proj):
    import ml_dtypes
    f = np.float32
    bf = ml_dtypes.bfloat16
    xb = np.ascontiguousarray(np.asarray(x, f).reshape(NCORES, C, N))
    wqkvT = np.ascontiguousarray(np.asarray(w_qkv, f).T.astype(bf))     # [C, 3C]
    wprojT = np.ascontiguousarray(np.asarray(w_proj, f).T.astype(bf))   # [C, C]
    bq = np.asarray(b_qkv, f)
    bqT = np.ascontiguousarray(bq[0:1024].reshape(8, 128).T)            # [128, 8]
    bpT = np.ascontiguousarray(np.asarray(b_proj, f).reshape(4, 128).T)  # [128, 4]
    bvb = np.ascontiguousarray(np.broadcast_to(bq[1024:1536][None, :], (128, 512)))
    gT = np.ascontiguousarray(np.asarray(gamma, f).reshape(4, 128).T)   # [128, 4]
    bT = np.ascontiguousarray(np.asarray(beta, f).reshape(4, 128).T)
    gsel = np.zeros((128, 4, NG), f)
    bsel = np.zeros((NG, C), f)
    for k in range(4):
        for p in range(128):
            g = 8 * k + p // 16
            gsel[p, k, g] = 1.0 / 16.0
            bsel[g, k * 128 + p] = 1.0
    onescol = np.ones((128, NH), bf)
    shared = {"wqkvT": wqkvT, "wprojT": wprojT, "bqT": bqT, "bpT": bpT, "bvb": bvb,
              "gammaT": gT, "betaT": bT, "gsel": gsel, "bsel": bsel, "onescol": onescol}
    return [dict(shared, x=xb[i]) for i in range(NCORES)]


def run(inputs, trace=False, **kwargs):
    from concourse.bass_utils import run_bass_kernel_spmd
    if "nc" not in _CACHE:
        _CACHE["nc"] = _build_program()
    nc = _CACHE["nc"]
    in_maps = _host_inputs(**inputs)
    res = run_bass_kernel_spmd(nc, in_maps, core_ids=list(range(NCORES)), trace=trace, **kwargs)
    H = W = 32
    y = np.stack([np.asarray(res.results[i]["y"], dtype=np.float32).reshape(C, H, W) for i in range(NCORES)])
    return y.astype(np.float32), res


def kernel(**inputs):
    y, _ = run(inputs, trace=False)
    return y
